# revision 5
# baseline (speedup 1.0000x reference)
"""Depthwise box-average (2r+1)x(2r+1), SAME zero padding, on trn2 x8 cores.

Input  x: [4, 32, 512, 512] f32, r = 4 (box 9x9, weights 1/81).
Output same shape.

Strategy (pure data parallel, no collectives):
  * Flatten N*C = 128 images of [512, 512]; core c takes images [16c, 16c+16).
  * Separable filter per image:
      vertical 9-tap:  TensorE banded-Toeplitz matmul (weights 1/81) into PSUM.
        Image rows are split into 4 chunks of 128 (the partition dim).  The
        band crossing chunk boundaries is handled with two sliver matmuls
        (U = last r rows of previous chunk, L = first r rows of next chunk)
        accumulating into the same PSUM bank.  Zero 'SAME' padding at image
        top/bottom is automatic (the band just truncates).
      horizontal 9-tap: sliding-window recurrence
          s[u] = s[u-1] + y[u+r] - y[u-r-1]
        done in one VectorE tensor_tensor_scan per row-chunk (plus two tiny
        edge scans reading a zero tile), after ScalarE evacuates PSUM->SBUF.
  * One 1MiB DMA in and one 1MiB DMA out per image.
"""

import numpy as np

import concourse.bacc as bacc
import concourse.bass as bass
import concourse.mybir as mybir
from concourse.bass_utils import run_bass_kernel_spmd
from concourse.tile import TileContext

N, C, H, W = 4, 32, 512, 512
P = 128                      # SBUF partitions / chunk height
NCORES = 8
IMGS = N * C                 # 128 images total
IMGS_PER_CORE = IMGS // NCORES

F32 = mybir.dt.float32
ADD = mybir.AluOpType.add
SUB = mybir.AluOpType.subtract

# set by test harness to capture a profile
TRACE = False
LAST_EXEC_NS = None
LAST_RESULTS = None


def _bands(r: int):
    """Banded Toeplitz blocks for the vertical pass, pre-scaled by 1/(2r+1)^2."""
    k = 2 * r + 1
    w = np.float32(1.0 / (k * k))
    i = np.arange(P)
    # D[i, m] : contribution of in-chunk row i to output row m of same chunk
    D = (np.abs(i[:, None] - i[None, :]) <= r).astype(np.float32) * w
    # U[i, m] : row (chunk_start - r + i), i in [0, r)  ->  output row m
    iu = np.arange(r)
    Uc = (np.abs((iu[:, None] - r) - i[None, :]) <= r).astype(np.float32) * w
    # L[i, m] : row (chunk_end + i), i in [0, r)  ->  output row m
    Lc = (np.abs((P + iu[:, None]) - i[None, :]) <= r).astype(np.float32) * w
    return D, Uc, Lc


def _build(r: int, n_imgs: int):
    k = 2 * r + 1
    chunks = H // P
    nc = bacc.Bacc("TRN2", target_bir_lowering=False, debug=False,
                   num_devices=NCORES)

    xs = nc.dram_tensor("xs", [n_imgs, H, W], F32, kind="ExternalInput").ap()
    bD = nc.dram_tensor("bD", [P, P], F32, kind="ExternalInput").ap()
    bU = nc.dram_tensor("bU", [r, P], F32, kind="ExternalInput").ap()
    bL = nc.dram_tensor("bL", [r, P], F32, kind="ExternalInput").ap()
    out = nc.dram_tensor("out", [n_imgs, H, W], F32, kind="ExternalOutput").ap()

    OW = W + r  # output tile row width: r leading pad cols + W data cols

    with TileContext(nc) as tc:
        with (
            tc.tile_pool(name="const", bufs=1) as cpool,
            tc.tile_pool(name="x", bufs=3) as xpool,
            tc.tile_pool(name="ys", bufs=4) as ypool,
            tc.tile_pool(name="o", bufs=3) as opool,
            tc.tile_pool(name="ps", bufs=4, space="PSUM") as ppool,
        ):
            bD_t = cpool.tile([P, P], F32)
            nc.sync.dma_start(out=bD_t[:], in_=bD)
            bU_t = cpool.tile([r, P], F32)
            nc.sync.dma_start(out=bU_t[:], in_=bU)
            bL_t = cpool.tile([r, P], F32)
            nc.sync.dma_start(out=bL_t[:], in_=bL)
            zt = cpool.tile([P, k], F32)
            nc.vector.memset(zt[:], 0.0)

            nb = chunks - 1  # chunk boundaries
            for img in range(n_imgs):
                xt = xpool.tile([P, chunks, W], F32, tag="x")
                nc.sync.dma_start(
                    out=xt[:],
                    in_=xs[img].rearrange("(s p) w -> p s w", p=P),
                )
                # sliver rows at the chunk boundaries, re-loaded at base
                # partition 0 (matmul operands must start at partition 0/32/64)
                svU = xpool.tile([r, nb, W], F32, tag="svU")
                nc.sync.dma_start(
                    out=svU[:],
                    in_=xs[img][P - r:P - r + nb * P]
                        .rearrange("(b q) w -> q b w", q=P)[0:r],
                )
                svL = xpool.tile([r, nb, W], F32, tag="svL")
                nc.sync.dma_start(
                    out=svL[:],
                    in_=xs[img][P:P + nb * P]
                        .rearrange("(b q) w -> q b w", q=P)[0:r],
                )
                ot = opool.tile([P, chunks, OW], F32, tag="o")
                for j in range(chunks):
                    y = ppool.tile([P, W], F32, tag="y")
                    n_mm = 1 + (j > 0) + (j < chunks - 1)
                    nc.tensor.matmul(y[:], bD_t[:], xt[:, j, :],
                                     start=True, stop=(n_mm == 1))
                    done = 1
                    if j > 0:
                        done += 1
                        nc.tensor.matmul(y[:], bU_t[:], svU[:, j - 1, :],
                                         start=False, stop=(done == n_mm))
                    if j < chunks - 1:
                        done += 1
                        nc.tensor.matmul(y[:], bL_t[:], svL[:, j, :],
                                         start=False, stop=(done == n_mm))

                    ys = ypool.tile([P, W], F32, tag="ys")
                    nc.scalar.copy(out=ys[:], in_=y[:])

                    oj = ot[:, j, :]
                    # s[u] = s[u-1] + y[u+r] - y[u-r-1]; col = u + r
                    # seg A: u in [-r, r]  -> cols [0, 2r]
                    nc.vector.tensor_tensor_scan(
                        oj[:, 0:k], ys[:, 0:k], zt[:, 0:k], 0.0, ADD, SUB)
                    # seg B: u in [r+1, W-r-1] -> cols [2r+1, W-1]
                    nc.vector.tensor_tensor_scan(
                        oj[:, k:W], ys[:, k:W], ys[:, 0:W - k],
                        oj[:, k - 1:k], ADD, SUB)
                    # seg C: u in [W-r, W-1] -> cols [W, W+r-1]
                    nc.vector.tensor_tensor_scan(
                        oj[:, W:W + r], zt[:, 0:r], ys[:, W - k:W - k + r],
                        oj[:, W - 1:W], ADD, SUB)

                nc.sync.dma_start(
                    out=out[img].rearrange("(s p) w -> p s w", p=P),
                    in_=ot[:, :, r:r + W],
                )
    nc.compile()
    return nc


def kernel(x, r):
    global LAST_EXEC_NS, LAST_RESULTS
    x = np.ascontiguousarray(np.asarray(x, dtype=np.float32))
    r = int(r)
    assert x.shape == (N, C, H, W)
    assert 1 <= r < P and H % P == 0

    D, U, L = _bands(r)
    nc = _build(r, IMGS_PER_CORE)

    shards = x.reshape(NCORES, IMGS_PER_CORE, H, W)
    in_maps = [
        {"xs": np.ascontiguousarray(shards[c]), "bD": D, "bU": U, "bL": L}
        for c in range(NCORES)
    ]
    res = run_bass_kernel_spmd(
        nc, in_maps, core_ids=list(range(NCORES)), trace=TRACE)
    LAST_EXEC_NS = res.exec_time_ns
    LAST_RESULTS = res
    outs = np.stack([res.results[c]["out"] for c in range(NCORES)], axis=0)
    return outs.reshape(N, C, H, W)


# revision 13
# speedup vs baseline: 1.3436x; 1.3436x over previous
"""Depthwise box-average (2r+1)x(2r+1), SAME zero padding, on trn2 x8 cores.

Input  x: [4, 32, 512, 512] f32, r = 4 (box 9x9, weights 1/81).
Output same shape.

Strategy (pure data parallel, no collectives):
  * Flatten N*C = 128 images of [512, 512]; core c takes images [16c, 16c+16).
  * Separable filter per image:
      vertical 9-tap:  TensorE banded-Toeplitz matmul (weights 1/81) into PSUM.
        Image rows are split into 4 chunks of 128 (the partition dim).  The
        band crossing chunk boundaries is handled with two sliver matmuls
        (U = last r rows of previous chunk, L = first r rows of next chunk)
        accumulating into the same PSUM bank.  Zero 'SAME' padding at image
        top/bottom is automatic (the band just truncates).
      horizontal 9-tap: sliding-window recurrence
          s[u] = s[u-1] + y[u+r] - y[u-r-1]
        done in one VectorE tensor_tensor_scan per row-chunk (plus two tiny
        edge scans reading a zero tile), after ScalarE evacuates PSUM->SBUF.
  * One 1MiB DMA in and one 1MiB DMA out per image.
"""

import numpy as np

import concourse.bacc as bacc
import concourse.bass as bass
import concourse.mybir as mybir
from concourse.bass_utils import run_bass_kernel_spmd
from concourse.tile import TileContext

N, C, H, W = 4, 32, 512, 512
P = 128                      # SBUF partitions / chunk height
NCORES = 8
IMGS = N * C                 # 128 images total
IMGS_PER_CORE = IMGS // NCORES

F32 = mybir.dt.float32
F32R = mybir.dt.float32r   # TF32-like PE mode: full-rate fp32 matmul, ~1e-6 err
ADD = mybir.AluOpType.add
SUB = mybir.AluOpType.subtract

# set by test harness to capture a profile
TRACE = False
LAST_EXEC_NS = None
LAST_RESULTS = None


def _bands(r: int):
    """Banded Toeplitz blocks for the vertical pass, pre-scaled by 1/(2r+1)^2."""
    k = 2 * r + 1
    w = np.float32(1.0 / (k * k))
    i = np.arange(P)
    # D[i, m] : contribution of in-chunk row i to output row m of same chunk
    D = (np.abs(i[:, None] - i[None, :]) <= r).astype(np.float32) * w
    # U[i, m] : row (chunk_start - r + i), i in [0, r)  ->  output row m
    iu = np.arange(r)
    Uc = (np.abs((iu[:, None] - r) - i[None, :]) <= r).astype(np.float32) * w
    # L[i, m] : row (chunk_end + i), i in [0, r)  ->  output row m
    Lc = (np.abs((P + iu[:, None]) - i[None, :]) <= r).astype(np.float32) * w
    return D, Uc, Lc


def _build(r: int, n_imgs: int):
    k = 2 * r + 1
    chunks = H // P
    nc = bacc.Bacc("TRN2", target_bir_lowering=False, debug=False,
                   num_devices=NCORES)

    xs = nc.dram_tensor("xs", [n_imgs, H, W], F32R, kind="ExternalInput").ap()
    bD = nc.dram_tensor("bD", [P, P], F32R, kind="ExternalInput").ap()
    bU = nc.dram_tensor("bU", [r, P], F32R, kind="ExternalInput").ap()
    bL = nc.dram_tensor("bL", [r, P], F32R, kind="ExternalInput").ap()
    out = nc.dram_tensor("out", [n_imgs, H, W], F32, kind="ExternalOutput").ap()

    OW = W + r  # output tile row width: r leading pad cols + W data cols

    with TileContext(nc) as tc:
        with (
            tc.tile_pool(name="const", bufs=1) as cpool,
            tc.tile_pool(name="x", bufs=3) as xpool,
            tc.tile_pool(name="ys", bufs=4) as ypool,
            tc.tile_pool(name="o", bufs=3) as opool,
            tc.tile_pool(name="ps", bufs=4, space="PSUM") as ppool,
        ):
            bD_t = cpool.tile([P, P], F32R)
            nc.sync.dma_start(out=bD_t[:], in_=bD)
            bU_t = cpool.tile([r, P], F32R)
            nc.sync.dma_start(out=bU_t[:], in_=bU)
            bL_t = cpool.tile([r, P], F32R)
            nc.sync.dma_start(out=bL_t[:], in_=bL)
            zt = cpool.tile([P, k], F32)
            nc.vector.memset(zt[:], 0.0)

            nb = chunks - 1  # chunk boundaries
            for img in range(n_imgs):
                xt = xpool.tile([P, chunks, W], F32R, tag="x")
                nc.sync.dma_start(
                    out=xt[:],
                    in_=xs[img].rearrange("(s p) w -> p s w", p=P),
                )
                # sliver rows at the chunk boundaries, re-loaded at base
                # partition 0 (matmul operands must start at partition 0/32/64)
                svU = xpool.tile([r, nb, W], F32R, tag="svU")
                nc.sync.dma_start(
                    out=svU[:],
                    in_=xs[img][P - r:P - r + nb * P]
                        .rearrange("(b q) w -> q b w", q=P)[0:r],
                )
                svL = xpool.tile([r, nb, W], F32R, tag="svL")
                nc.sync.dma_start(
                    out=svL[:],
                    in_=xs[img][P:P + nb * P]
                        .rearrange("(b q) w -> q b w", q=P)[0:r],
                )
                ot = opool.tile([P, chunks, OW], F32, tag="o")
                for j in range(chunks):
                    y = ppool.tile([P, W], F32, tag="y")
                    n_mm = 1 + (j > 0) + (j < chunks - 1)
                    nc.tensor.matmul(y[:], bD_t[:], xt[:, j, :],
                                     start=True, stop=(n_mm == 1))
                    done = 1
                    if j > 0:
                        done += 1
                        nc.tensor.matmul(y[:], bU_t[:], svU[:, j - 1, :],
                                         start=False, stop=(done == n_mm))
                    if j < chunks - 1:
                        done += 1
                        nc.tensor.matmul(y[:], bL_t[:], svL[:, j, :],
                                         start=False, stop=(done == n_mm))

                    ys = ypool.tile([P, W], F32, tag="ys")
                    nc.scalar.copy(out=ys[:], in_=y[:])

                    oj = ot[:, j, :]
                    # s[u] = s[u-1] + y[u+r] - y[u-r-1]; col = u + r
                    # seg A: u in [-r, r]  -> cols [0, 2r]
                    nc.vector.tensor_tensor_scan(
                        oj[:, 0:k], ys[:, 0:k], zt[:, 0:k], 0.0, ADD, SUB)
                    # seg B: u in [r+1, W-r-1] -> cols [2r+1, W-1]
                    nc.vector.tensor_tensor_scan(
                        oj[:, k:W], ys[:, k:W], ys[:, 0:W - k],
                        oj[:, k - 1:k], ADD, SUB)
                    # seg C: u in [W-r, W-1] -> cols [W, W+r-1]
                    nc.vector.tensor_tensor_scan(
                        oj[:, W:W + r], zt[:, 0:r], ys[:, W - k:W - k + r],
                        oj[:, W - 1:W], ADD, SUB)

                nc.sync.dma_start(
                    out=out[img].rearrange("(s p) w -> p s w", p=P),
                    in_=ot[:, :, r:r + W],
                )
    nc.compile()
    return nc


def kernel(x, r):
    global LAST_EXEC_NS, LAST_RESULTS
    x = np.ascontiguousarray(np.asarray(x, dtype=np.float32))
    r = int(r)
    assert x.shape == (N, C, H, W)
    assert 1 <= r < P and H % P == 0

    D, U, L = _bands(r)
    nc = _build(r, IMGS_PER_CORE)

    shards = x.reshape(NCORES, IMGS_PER_CORE, H, W)
    in_maps = [
        {"xs": np.ascontiguousarray(shards[c]), "bD": D, "bU": U, "bL": L}
        for c in range(NCORES)
    ]
    res = run_bass_kernel_spmd(
        nc, in_maps, core_ids=list(range(NCORES)), trace=TRACE)
    LAST_EXEC_NS = res.exec_time_ns
    LAST_RESULTS = res
    outs = np.stack([res.results[c]["out"] for c in range(NCORES)], axis=0)
    return outs.reshape(N, C, H, W)


# revision 18
# speedup vs baseline: 1.9535x; 1.4539x over previous
"""Depthwise box-average (2r+1)x(2r+1), SAME zero padding, on trn2 x8 cores.

Input  x: [4, 32, 512, 512] f32, r = 4 (box 9x9, weights 1/81).
Output same shape.

Strategy (pure data parallel, no collectives):
  * Flatten N*C = 128 images of [512, 512]; core c takes images [16c, 16c+16).
  * Separable filter per image:
      vertical 9-tap:  TensorE banded-Toeplitz matmul (weights 1/81) into PSUM.
        Image rows are split into 4 chunks of 128 (the partition dim).  The
        band crossing chunk boundaries is handled with two sliver matmuls
        (U = last r rows of previous chunk, L = first r rows of next chunk)
        accumulating into the same PSUM bank.  Zero 'SAME' padding at image
        top/bottom is automatic (the band just truncates).
      horizontal 9-tap: sliding-window recurrence
          s[u] = s[u-1] + y[u+r] - y[u-r-1]
        done in one VectorE tensor_tensor_scan per row-chunk (plus two tiny
        edge scans reading a zero tile), after ScalarE evacuates PSUM->SBUF.
  * One 1MiB DMA in and one 1MiB DMA out per image.
"""

import numpy as np

import concourse.bacc as bacc
import concourse.bass as bass
import concourse.mybir as mybir
from concourse.bass_utils import run_bass_kernel_spmd
from concourse.tile import TileContext

N, C, H, W = 4, 32, 512, 512
P = 128                      # SBUF partitions / chunk height
NCORES = 8
IMGS = N * C                 # 128 images total
IMGS_PER_CORE = IMGS // NCORES

F32 = mybir.dt.float32
F32R = mybir.dt.float32r   # TF32-like PE mode: full-rate fp32 matmul, ~1e-6 err
ADD = mybir.AluOpType.add
SUB = mybir.AluOpType.subtract

# set by test harness to capture a profile
TRACE = False
LAST_EXEC_NS = None
LAST_RESULTS = None


def _bands(r: int):
    """Banded Toeplitz blocks for the vertical pass, pre-scaled by 1/(2r+1)^2."""
    k = 2 * r + 1
    w = np.float32(1.0 / (k * k))
    i = np.arange(P)
    # D[i, m] : contribution of in-chunk row i to output row m of same chunk
    D = (np.abs(i[:, None] - i[None, :]) <= r).astype(np.float32) * w
    # U[i, m] : row (chunk_start - r + i), i in [0, r)  ->  output row m
    iu = np.arange(r)
    Uc = (np.abs((iu[:, None] - r) - i[None, :]) <= r).astype(np.float32) * w
    # L[i, m] : row (chunk_end + i), i in [0, r)  ->  output row m
    Lc = (np.abs((P + iu[:, None]) - i[None, :]) <= r).astype(np.float32) * w
    return D, Uc, Lc


def _build(r: int, n_imgs: int):
    k = 2 * r + 1
    chunks = H // P
    nc = bacc.Bacc("TRN2", target_bir_lowering=False, debug=False,
                   num_devices=NCORES)

    xs = nc.dram_tensor("xs", [n_imgs, H, W], F32R, kind="ExternalInput").ap()
    bD = nc.dram_tensor("bD", [P, P], F32R, kind="ExternalInput").ap()
    bU = nc.dram_tensor("bU", [r, P], F32R, kind="ExternalInput").ap()
    bL = nc.dram_tensor("bL", [r, P], F32R, kind="ExternalInput").ap()
    out = nc.dram_tensor("out", [n_imgs, H, W], F32, kind="ExternalOutput").ap()

    OW = W + r  # output tile row width: r leading pad cols + W data cols

    with TileContext(nc) as tc:
        with (
            tc.tile_pool(name="const", bufs=1) as cpool,
            tc.tile_pool(name="x", bufs=4) as xpool,
            tc.tile_pool(name="ys", bufs=6) as ypool,
            tc.tile_pool(name="o", bufs=5) as opool,
            tc.tile_pool(name="ps", bufs=6, space="PSUM") as ppool,
        ):
            bD_t = cpool.tile([P, P], F32R)
            nc.sync.dma_start(out=bD_t[:], in_=bD)
            bU_t = cpool.tile([r, P], F32R)
            nc.sync.dma_start(out=bU_t[:], in_=bU)
            bL_t = cpool.tile([r, P], F32R)
            nc.sync.dma_start(out=bL_t[:], in_=bL)
            zt = cpool.tile([P, k], F32)
            nc.vector.memset(zt[:], 0.0)

            nb = chunks - 1  # chunk boundaries
            for img in range(n_imgs):
                xt = xpool.tile([P, chunks, W], F32R, tag="x")
                nc.sync.dma_start(
                    out=xt[:],
                    in_=xs[img].rearrange("(s p) w -> p s w", p=P),
                )
                # sliver rows at the chunk boundaries, re-loaded at base
                # partition 0 (matmul operands must start at partition 0/32/64):
                # sv[p, b, h, :] = row (P - r + P*b + r*h + p); h=0 -> last r
                # rows of chunk b (U), h=1 -> first r rows of chunk b+1 (L)
                sv = xpool.tile([r, nb, 2, W], F32R, tag="sv")
                nc.sync.dma_start(
                    out=sv[:, :, 0, :],
                    in_=xs[img][P - r:P - r + nb * P]
                        .rearrange("(b q) w -> q b w", q=P)[0:r],
                )
                nc.sync.dma_start(
                    out=sv[:, :, 1, :],
                    in_=xs[img][P:P + nb * P]
                        .rearrange("(b q) w -> q b w", q=P)[0:r],
                )
                ot = opool.tile([P, chunks, OW], F32, tag="o")
                for j in range(chunks):
                    y = ppool.tile([P, W], F32, tag="y")
                    n_mm = 1 + (j > 0) + (j < chunks - 1)
                    nc.tensor.matmul(y[:], bD_t[:], xt[:, j, :],
                                     start=True, stop=(n_mm == 1))
                    done = 1
                    if j > 0:
                        done += 1
                        nc.tensor.matmul(y[:], bU_t[:], sv[:, j - 1, 0, :],
                                         start=False, stop=(done == n_mm))
                    if j < chunks - 1:
                        done += 1
                        nc.tensor.matmul(y[:], bL_t[:], sv[:, j, 1, :],
                                         start=False, stop=(done == n_mm))

                    ys = ypool.tile([P, W], F32, tag="ys")
                    nc.scalar.copy(out=ys[:], in_=y[:])

                    oj = ot[:, j, :]
                    # s[u] = s[u-1] + y[u+r] - y[u-r-1]; col = u + r
                    # seg A: u in [-r, r]  -> cols [0, 2r]
                    nc.vector.tensor_tensor_scan(
                        oj[:, 0:k], ys[:, 0:k], zt[:, 0:k], 0.0, ADD, SUB)
                    # seg B: u in [r+1, W-r-1] -> cols [2r+1, W-1]
                    nc.vector.tensor_tensor_scan(
                        oj[:, k:W], ys[:, k:W], ys[:, 0:W - k],
                        oj[:, k - 1:k], ADD, SUB)
                    # seg C: u in [W-r, W-1] -> cols [W, W+r-1]
                    nc.vector.tensor_tensor_scan(
                        oj[:, W:W + r], zt[:, 0:r], ys[:, W - k:W - k + r],
                        oj[:, W - 1:W], ADD, SUB)

                # out-DMA on the (otherwise idle) gpsimd SWDGE queue so its
                # semaphore wait doesn't head-of-line-block the input DMAs
                # streaming on the sync queue.
                nc.gpsimd.dma_start(
                    out=out[img].rearrange("(s p) w -> p s w", p=P),
                    in_=ot[:, :, r:r + W],
                )
    nc.compile()
    return nc


def kernel(x, r):
    global LAST_EXEC_NS, LAST_RESULTS
    x = np.ascontiguousarray(np.asarray(x, dtype=np.float32))
    r = int(r)
    assert x.shape == (N, C, H, W)
    assert 1 <= r < P and H % P == 0

    D, U, L = _bands(r)
    nc = _build(r, IMGS_PER_CORE)

    shards = x.reshape(NCORES, IMGS_PER_CORE, H, W)
    in_maps = [
        {"xs": np.ascontiguousarray(shards[c]), "bD": D, "bU": U, "bL": L}
        for c in range(NCORES)
    ]
    res = run_bass_kernel_spmd(
        nc, in_maps, core_ids=list(range(NCORES)), trace=TRACE)
    LAST_EXEC_NS = res.exec_time_ns
    LAST_RESULTS = res
    outs = np.stack([res.results[c]["out"] for c in range(NCORES)], axis=0)
    return outs.reshape(N, C, H, W)
